# revision 5
# baseline (speedup 1.0000x reference)
"""AlphaCompositor Trainium2 kernel (8 NeuronCores, data-parallel over N).

Per core: one image n. out[c,h,w] = sum_k w_k * ptclds[c, idx_k], with
w_k = a_k * prod_{j<k}(1-a_j), a = alpha * (idx >= 0); background pixels
(idx[0] < 0) get color [0,0,0,1]; second output is the foreground mask.

Gather strategy: GPSIMD ap_gather from an SBUF-resident table.
 - Table [4, 100000] f32 is laid out as [128, 25000]: SBUF row 16g+4q+ch
   holds channel ch of table chunk q (25000 points per chunk); identical
   for each of the 8 16-partition groups g.
 - GPSIMD group g gathers, for every pixel j of a batch, the k=g point
   index (within-chunk offset r = idx - 25000q). One gather event yields
   all 16 (q, ch) candidate values for that (k=g, pixel) pair.
 - PE transposes the candidate tile to pixel-major [pixel, (k,q,ch)];
   DVE masks the correct chunk (q == idx//25000), applies blend weights,
   and reduces over (k, q).
"""

import sys
sys.path.insert(0, "/opt/trn_rl_repo")

import numpy as np

N, K, H, W, C, P = 8, 8, 512, 512, 4, 100000
HW = H * W                # 262144 pixels per image
CHUNK = 25000             # table points per chunk (4 chunks)
NB = 4096                 # pixels per batch
NBATCH = HW // NB         # 64
BLK = NB // 128           # 32 pixel-blocks of 128 per batch
NB16 = NB // 16           # wrapped idx columns

_NC_CACHE = None


def _build_nc():
    from concourse import bacc, bass, mybir, tile
    from concourse.masks import make_identity

    f32 = mybir.dt.float32
    i32 = mybir.dt.int32
    i16 = mybir.dt.int16
    op = mybir.AluOpType

    nc = bacc.Bacc("TRN2", target_bir_lowering=False, debug=False, num_devices=8)
    tab_d = nc.dram_tensor("tab", [128, CHUNK], f32, kind="ExternalInput")
    idxw_d = nc.dram_tensor("idxw", [NBATCH, 128, NB16], i32, kind="ExternalInput")
    idxp_d = nc.dram_tensor("idxp", [NBATCH, 128, BLK * 8], i32, kind="ExternalInput")
    alp_d = nc.dram_tensor("alp", [NBATCH, 128, BLK * 8], f32, kind="ExternalInput")
    img_d = nc.dram_tensor("img", [NBATCH, 128, BLK * 4], f32, kind="ExternalOutput")
    fg_d = nc.dram_tensor("fg", [NBATCH, 128, BLK], f32, kind="ExternalOutput")

    with tile.TileContext(nc) as tc:
        with tc.tile_pool(name="tabp", bufs=1) as tabp, \
             tc.tile_pool(name="inp", bufs=4) as inp, \
             tc.tile_pool(name="wk", bufs=2) as wk, \
             tc.tile_pool(name="gp", bufs=2) as gp, \
             tc.tile_pool(name="xp", bufs=2) as xp, \
             tc.tile_pool(name="mp", bufs=2) as mp, \
             tc.tile_pool(name="outp", bufs=4) as outp, \
             tc.tile_pool(name="pp", bufs=6, space="PSUM") as pp:

            tab = tabp.tile([128, CHUNK], f32)
            nc.sync.dma_start(tab[:], tab_d.ap())
            ident = tabp.tile([128, 128], f32)
            make_identity(nc, ident[:])

            for b in range(NBATCH):
                # ---- wrapped idx -> within-chunk int16 offsets ----------
                iw = inp.tile([128, NB16], i32, tag="iw")
                nc.sync.dma_start(iw[:], idxw_d.ap()[b])
                qw = wk.tile([128, NB16], i32, tag="qw")
                tq = wk.tile([128, NB16], i32, tag="tq")
                nc.vector.tensor_scalar(out=qw[:], in0=iw[:], scalar1=CHUNK,
                                        scalar2=None, op0=op.is_ge)
                nc.vector.tensor_scalar(out=tq[:], in0=iw[:], scalar1=2 * CHUNK,
                                        scalar2=None, op0=op.is_ge)
                nc.vector.tensor_tensor(out=qw[:], in0=qw[:], in1=tq[:], op=op.add)
                nc.vector.tensor_scalar(out=tq[:], in0=iw[:], scalar1=3 * CHUNK,
                                        scalar2=None, op0=op.is_ge)
                nc.vector.tensor_tensor(out=qw[:], in0=qw[:], in1=tq[:], op=op.add)
                nc.vector.tensor_scalar(out=qw[:], in0=qw[:], scalar1=CHUNK,
                                        scalar2=None, op0=op.mult)
                nc.vector.tensor_tensor(out=qw[:], in0=iw[:], in1=qw[:], op=op.subtract)
                r16 = wk.tile([128, NB16], i16, tag="r16")
                nc.vector.tensor_copy(out=r16[:], in_=qw[:])

                # ---- gather: group g fetches k=g candidates per pixel ---
                cand = gp.tile([128, NB], f32, tag="cand")
                nc.gpsimd.ap_gather(
                    out_ap=cand[:], in_ap=tab[:], idxs_ap=r16[:],
                    channels=128, num_elems=CHUNK, d=1, num_idxs=NB,
                )

                # ---- transpose to pixel-major X[p, t*128 + (k*16+q*4+c)] -
                X = xp.tile([128, NB], f32, tag="X")
                for t in range(BLK):
                    ps = pp.tile([128, 128], f32, tag="ps")
                    nc.tensor.transpose(
                        out=ps[:], in_=cand[:, t * 128:(t + 1) * 128],
                        identity=ident[:])
                    nc.vector.tensor_copy(out=X[:, t * 128:(t + 1) * 128], in_=ps[:])

                # ---- pixel-major idx / alphas --------------------------
                ip = inp.tile([128, BLK * 8], i32, tag="ip")
                nc.sync.dma_start(ip[:], idxp_d.ap()[b])
                al = inp.tile([128, BLK * 8], f32, tag="al")
                nc.sync.dma_start(al[:], alp_d.ap()[b])

                # chunk id per (pixel, k): qp in {0,1,2,3} (int32)
                qp = wk.tile([128, BLK * 8], i32, tag="qp")
                tp = wk.tile([128, BLK * 8], i32, tag="tp")
                nc.vector.tensor_scalar(out=qp[:], in0=ip[:], scalar1=CHUNK,
                                        scalar2=None, op0=op.is_ge)
                nc.vector.tensor_scalar(out=tp[:], in0=ip[:], scalar1=2 * CHUNK,
                                        scalar2=None, op0=op.is_ge)
                nc.vector.tensor_tensor(out=qp[:], in0=qp[:], in1=tp[:], op=op.add)
                nc.vector.tensor_scalar(out=tp[:], in0=ip[:], scalar1=3 * CHUNK,
                                        scalar2=None, op0=op.is_ge)
                nc.vector.tensor_tensor(out=qp[:], in0=qp[:], in1=tp[:], op=op.add)

                # blend weights w[p, j*8+k]
                valid = wk.tile([128, BLK * 8], f32, tag="valid")
                nc.vector.tensor_scalar(out=valid[:], in0=ip[:], scalar1=0,
                                        scalar2=None, op0=op.is_ge)
                a = wk.tile([128, BLK * 8], f32, tag="a")
                nc.vector.tensor_tensor(out=a[:], in0=al[:], in1=valid[:], op=op.mult)
                wgt = wk.tile([128, BLK * 8], f32, tag="wgt")
                trans = wk.tile([128, BLK], f32, tag="trans")
                nc.vector.memset(trans[:], 1.0)
                a3 = a[:].rearrange("p (j k) -> p j k", k=8)
                w3 = wgt[:].rearrange("p (j k) -> p j k", k=8)
                t3 = trans[:].rearrange("p (j o) -> p j o", o=1)
                for k in range(8):
                    nc.vector.tensor_tensor(
                        out=w3[:, :, k:k + 1], in0=a3[:, :, k:k + 1],
                        in1=t3, op=op.mult)
                    nc.vector.tensor_tensor(
                        out=t3, in0=t3, in1=w3[:, :, k:k + 1], op=op.subtract)

                # masked weights M[p, j*32 + k*4 + q] = w * (qp == q)
                M = mp.tile([128, BLK * 32], f32, tag="M")
                cm = wk.tile([128, BLK * 8], f32, tag="cm")
                m4 = M[:].rearrange("p (j k q) -> p j k q", k=8, q=4)
                for q in range(4):
                    nc.vector.tensor_scalar(out=cm[:], in0=qp[:], scalar1=q,
                                            scalar2=None, op0=op.is_equal)
                    nc.vector.tensor_tensor(
                        out=m4[:, :, :, q:q + 1].rearrange("p j k o -> p j (k o)"),
                        in0=cm[:].rearrange("p (j k) -> p j k", k=8),
                        in1=wgt[:].rearrange("p (j k) -> p j k", k=8),
                        op=op.mult)

                # ---- combine: acc[p, j*4+c] = sum_t M[p,j,t] * X[p,j*128+t*4+c]
                acc = outp.tile([128, BLK * 4], f32, tag="acc")
                tmp = outp.tile([128, BLK * 4], f32, tag="tmp")
                accv = acc[:].rearrange("p (j c) -> p j c", c=4)
                tmpv = tmp[:].rearrange("p (j c) -> p j c", c=4)
                xv = X[:].rearrange("p (j r) -> p j r", r=128)
                mv = M[:].rearrange("p (j t) -> p j t", t=32)
                nc.vector.memset(acc[:], 0.0)
                for t in range(32):
                    mb = mv[:, :, t:t + 1].to_broadcast([128, BLK, 4])
                    nc.any.tensor_tensor(
                        out=tmpv, in0=xv[:, :, t * 4:t * 4 + 4], in1=mb,
                        op=op.mult)
                    nc.any.tensor_tensor(out=accv, in0=accv, in1=tmpv, op=op.add)

                # background: alpha channel += (idx0 < 0)
                bgf = wk.tile([128, BLK], f32, tag="bgf")
                ip0 = ip[:].rearrange("p (j k) -> p j k", k=8)[:, :, 0:1]
                nc.vector.tensor_scalar(
                    out=bgf[:].rearrange("p (j o) -> p j o", o=1), in0=ip0, scalar1=0,
                    scalar2=None, op0=op.is_lt)
                nc.vector.tensor_tensor(
                    out=accv[:, :, 3:4], in0=accv[:, :, 3:4],
                    in1=bgf[:].rearrange("p (j o) -> p j o", o=1), op=op.add)

                # foreground mask output
                fgt = outp.tile([128, BLK], f32, tag="fgt")
                nc.vector.tensor_scalar(
                    out=fgt[:].rearrange("p (j o) -> p j o", o=1), in0=ip0, scalar1=0,
                    scalar2=None, op0=op.is_ge)
                nc.sync.dma_start(fg_d.ap()[b], fgt[:])
                nc.sync.dma_start(img_d.ap()[b], acc[:])

    nc.compile()
    return nc


def _get_nc():
    global _NC_CACHE
    if _NC_CACHE is None:
        _NC_CACHE = _build_nc()
    return _NC_CACHE


def _prep_core_inputs(pix_idxs_n, alphas_n, tab):
    """Host-side layout (pure permutation / dtype casts) for one image."""
    Pk = pix_idxs_n.reshape(K, HW).astype(np.int32)
    A = alphas_n.reshape(K, HW).astype(np.float32)
    # wrapped: idxw[b, 16g+s, c] = Pk[g, b*NB + c*16 + s]
    idxw = (
        Pk.reshape(K, NBATCH, NB16, 16)
        .transpose(1, 0, 3, 2)
        .reshape(NBATCH, 128, NB16)
    )
    # pixel-major: idxp[b, p, j*8+k] = Pk[k, b*NB + j*128 + p]
    idxp = (
        Pk.reshape(K, NBATCH, BLK, 128)
        .transpose(1, 3, 2, 0)
        .reshape(NBATCH, 128, BLK * 8)
    )
    alp = (
        A.reshape(K, NBATCH, BLK, 128)
        .transpose(1, 3, 2, 0)
        .reshape(NBATCH, 128, BLK * 8)
    )
    return {
        "tab": tab,
        "idxw": np.ascontiguousarray(idxw),
        "idxp": np.ascontiguousarray(idxp),
        "alp": np.ascontiguousarray(alp),
    }


def kernel(pix_idxs, alphas, ptclds):
    from concourse import bass_utils

    nc = _get_nc()
    T = np.asarray(ptclds, dtype=np.float32)
    # tab[16g + 4q + ch, e] = T[ch, q*CHUNK + e], identical per group g
    base = T.reshape(C, 4, CHUNK).transpose(1, 0, 2).reshape(16, CHUNK)
    tab = np.ascontiguousarray(np.tile(base, (8, 1)))

    pix_idxs = np.asarray(pix_idxs)
    alphas = np.asarray(alphas)
    in_maps = [
        _prep_core_inputs(pix_idxs[n], alphas[n], tab) for n in range(N)
    ]
    res = bass_utils.run_bass_kernel_spmd(nc, in_maps, core_ids=list(range(8)))

    images = np.empty((N, C, H, W), np.float32)
    masks = np.empty((N, H, W), bool)
    for n in range(N):
        img = res.results[n]["img"]  # [NBATCH, 128, BLK*4]
        fg = res.results[n]["fg"]    # [NBATCH, 128, BLK]
        img_full = (
            img.reshape(NBATCH, 128, BLK, C)
            .transpose(3, 0, 2, 1)
            .reshape(C, H, W)
        )
        images[n] = img_full
        masks[n] = (
            fg.reshape(NBATCH, 128, BLK)
            .transpose(0, 2, 1)
            .reshape(H, W)
        ) > 0.5
    return images, masks


# revision 6
# speedup vs baseline: 1.2303x; 1.2303x over previous
"""AlphaCompositor Trainium2 kernel (8 NeuronCores, data-parallel over N).

Per core: one image n. out[c,h,w] = sum_k w_k * ptclds[c, idx_k], with
w_k = a_k * prod_{j<k}(1-a_j), a = alpha * (idx >= 0); background pixels
(idx[0] < 0) get color [0,0,0,1]; second output is the foreground mask.

Gather strategy: GPSIMD ap_gather from an SBUF-resident table.
 - Table [4, 100000] f32 is laid out as [128, 25000]: SBUF row 16g+4q+ch
   holds channel ch of table chunk q (25000 points per chunk); identical
   for each of the 8 16-partition groups g.
 - GPSIMD group g gathers, for every pixel j of a batch, the k=g point
   index (within-chunk offset r = idx - 25000q). One gather event yields
   all 16 (q, ch) candidate values for that (k=g, pixel) pair.
 - PE transposes the candidate tile to pixel-major [pixel, (k,q,ch)];
   DVE masks the correct chunk (q == idx//25000), applies blend weights,
   and reduces over (k, q).
The build emits a 2-stage software pipeline (prep+gather of batch b before
select/combine of batch b-1) so the Pool engine never waits on DVE.
"""

import sys
sys.path.insert(0, "/opt/trn_rl_repo")

import numpy as np

N, K, H, W, C, P = 8, 8, 512, 512, 4, 100000
HW = H * W                # 262144 pixels per image
CHUNK = 25000             # table points per chunk (4 chunks)
NB = 4096                 # pixels per batch
NBATCH = HW // NB         # 64
BLK = NB // 128           # 32 pixel-blocks of 128 per batch
NB16 = NB // 16           # wrapped idx columns

_NC_CACHE = None


def _build_nc():
    from concourse import bacc, mybir, tile
    from concourse.masks import make_identity

    f32 = mybir.dt.float32
    i32 = mybir.dt.int32
    i16 = mybir.dt.int16
    op = mybir.AluOpType

    nc = bacc.Bacc("TRN2", target_bir_lowering=False, debug=False, num_devices=8)
    tab_d = nc.dram_tensor("tab", [128, CHUNK], f32, kind="ExternalInput")
    idxw_d = nc.dram_tensor("idxw", [NBATCH, 128, NB16], i32, kind="ExternalInput")
    idxp_d = nc.dram_tensor("idxp", [NBATCH, 128, BLK * 8], i32, kind="ExternalInput")
    alp_d = nc.dram_tensor("alp", [NBATCH, 128, BLK * 8], f32, kind="ExternalInput")
    img_d = nc.dram_tensor("img", [NBATCH, 128, BLK * 4], f32, kind="ExternalOutput")
    fg_d = nc.dram_tensor("fg", [NBATCH, 128, BLK], f32, kind="ExternalOutput")

    with tile.TileContext(nc) as tc:
        with tc.tile_pool(name="tabp", bufs=1) as tabp, \
             tc.tile_pool(name="inp", bufs=3) as inp, \
             tc.tile_pool(name="wk", bufs=2) as wk, \
             tc.tile_pool(name="gp", bufs=2) as gp, \
             tc.tile_pool(name="xp", bufs=2) as xp, \
             tc.tile_pool(name="mp", bufs=2) as mp, \
             tc.tile_pool(name="outp", bufs=3) as outp, \
             tc.tile_pool(name="pp", bufs=4, space="PSUM") as pp:

            tab = tabp.tile([128, CHUNK], f32)
            nc.sync.dma_start(tab[:], tab_d.ap())
            ident = tabp.tile([128, 128], f32)
            make_identity(nc, ident[:])

            def emit_prep_gather(b):
                """Stage 1: load wrapped idx, compute int16 chunk offsets,
                launch the gather; also start pixel-major input DMAs."""
                iw = inp.tile([128, NB16], i32, tag="iw")
                nc.sync.dma_start(iw[:], idxw_d.ap()[b])
                qw = wk.tile([128, NB16], i32, tag="qw")
                tq = wk.tile([128, NB16], i32, tag="tq")
                nc.vector.tensor_scalar(out=qw[:], in0=iw[:], scalar1=CHUNK,
                                        scalar2=None, op0=op.is_ge)
                nc.vector.tensor_scalar(out=tq[:], in0=iw[:], scalar1=2 * CHUNK,
                                        scalar2=None, op0=op.is_ge)
                nc.vector.tensor_tensor(out=qw[:], in0=qw[:], in1=tq[:], op=op.add)
                nc.vector.tensor_scalar(out=tq[:], in0=iw[:], scalar1=3 * CHUNK,
                                        scalar2=None, op0=op.is_ge)
                nc.vector.tensor_tensor(out=qw[:], in0=qw[:], in1=tq[:], op=op.add)
                nc.vector.tensor_scalar(out=qw[:], in0=qw[:], scalar1=CHUNK,
                                        scalar2=None, op0=op.mult)
                nc.vector.tensor_tensor(out=qw[:], in0=iw[:], in1=qw[:],
                                        op=op.subtract)
                r16 = wk.tile([128, NB16], i16, tag="r16")
                nc.vector.tensor_copy(out=r16[:], in_=qw[:])

                cand = gp.tile([128, NB], f32, tag="cand")
                nc.gpsimd.ap_gather(
                    out_ap=cand[:], in_ap=tab[:], idxs_ap=r16[:],
                    channels=128, num_elems=CHUNK, d=1, num_idxs=NB,
                )
                ip = inp.tile([128, BLK * 8], i32, tag="ip")
                nc.sync.dma_start(ip[:], idxp_d.ap()[b])
                al = inp.tile([128, BLK * 8], f32, tag="al")
                nc.sync.dma_start(al[:], alp_d.ap()[b])
                return cand, ip, al

            def emit_select(b, cand, ip, al):
                """Stage 2: transpose candidates, mask/weight, reduce, store."""
                X = xp.tile([128, NB], f32, tag="X")
                for t in range(BLK):
                    ps = pp.tile([128, 128], f32, tag="ps")
                    nc.tensor.transpose(
                        out=ps[:], in_=cand[:, t * 128:(t + 1) * 128],
                        identity=ident[:])
                    nc.vector.tensor_copy(out=X[:, t * 128:(t + 1) * 128],
                                          in_=ps[:])

                # chunk id per (pixel, k): qp in {0,1,2,3}
                qp = wk.tile([128, BLK * 8], i32, tag="qp")
                tp = wk.tile([128, BLK * 8], i32, tag="tp")
                nc.vector.tensor_scalar(out=qp[:], in0=ip[:], scalar1=CHUNK,
                                        scalar2=None, op0=op.is_ge)
                nc.vector.tensor_scalar(out=tp[:], in0=ip[:], scalar1=2 * CHUNK,
                                        scalar2=None, op0=op.is_ge)
                nc.vector.tensor_tensor(out=qp[:], in0=qp[:], in1=tp[:], op=op.add)
                nc.vector.tensor_scalar(out=tp[:], in0=ip[:], scalar1=3 * CHUNK,
                                        scalar2=None, op0=op.is_ge)
                nc.vector.tensor_tensor(out=qp[:], in0=qp[:], in1=tp[:], op=op.add)

                # blend weights w[p, j*8+k]
                valid = wk.tile([128, BLK * 8], f32, tag="valid")
                nc.vector.tensor_scalar(out=valid[:], in0=ip[:], scalar1=0,
                                        scalar2=None, op0=op.is_ge)
                a = wk.tile([128, BLK * 8], f32, tag="a")
                nc.vector.tensor_tensor(out=a[:], in0=al[:], in1=valid[:],
                                        op=op.mult)
                wgt = wk.tile([128, BLK * 8], f32, tag="wgt")
                trans = wk.tile([128, BLK], f32, tag="trans")
                nc.vector.memset(trans[:], 1.0)
                a3 = a[:].rearrange("p (j k) -> p j k", k=8)
                w3 = wgt[:].rearrange("p (j k) -> p j k", k=8)
                t3 = trans[:].rearrange("p (j o) -> p j o", o=1)
                for k in range(8):
                    nc.vector.tensor_tensor(
                        out=w3[:, :, k:k + 1], in0=a3[:, :, k:k + 1],
                        in1=t3, op=op.mult)
                    nc.vector.tensor_tensor(
                        out=t3, in0=t3, in1=w3[:, :, k:k + 1], op=op.subtract)

                # masked weights M[p, j*32 + k*4 + q] = w * (qp == q)
                M = mp.tile([128, BLK * 32], f32, tag="M")
                cm = wk.tile([128, BLK * 8], f32, tag="cm")
                m4 = M[:].rearrange("p (j k q) -> p j k q", k=8, q=4)
                for q in range(4):
                    nc.vector.tensor_scalar(out=cm[:], in0=qp[:], scalar1=q,
                                            scalar2=None, op0=op.is_equal)
                    nc.vector.tensor_tensor(
                        out=m4[:, :, :, q:q + 1].rearrange("p j k o -> p j (k o)"),
                        in0=cm[:].rearrange("p (j k) -> p j k", k=8),
                        in1=wgt[:].rearrange("p (j k) -> p j k", k=8),
                        op=op.mult)

                # combine: acc[p, j*4+c] = sum_t M[p,j,t] * X[p, j*128 + t*4+c]
                acc = outp.tile([128, BLK * 4], f32, tag="acc")
                tmp = outp.tile([128, BLK * 4], f32, tag="tmp")
                accv = acc[:].rearrange("p (j c) -> p j c", c=4)
                tmpv = tmp[:].rearrange("p (j c) -> p j c", c=4)
                xv = X[:].rearrange("p (j r) -> p j r", r=128)
                mv = M[:].rearrange("p (j t) -> p j t", t=32)
                nc.vector.memset(acc[:], 0.0)
                for t in range(32):
                    mb = mv[:, :, t:t + 1].to_broadcast([128, BLK, 4])
                    nc.any.tensor_tensor(
                        out=tmpv, in0=xv[:, :, t * 4:t * 4 + 4], in1=mb,
                        op=op.mult)
                    nc.any.tensor_tensor(out=accv, in0=accv, in1=tmpv, op=op.add)

                # background: alpha channel += (idx0 < 0)
                bgf = wk.tile([128, BLK], f32, tag="bgf")
                ip0 = ip[:].rearrange("p (j k) -> p j k", k=8)[:, :, 0:1]
                nc.vector.tensor_scalar(
                    out=bgf[:].rearrange("p (j o) -> p j o", o=1), in0=ip0,
                    scalar1=0, scalar2=None, op0=op.is_lt)
                nc.vector.tensor_tensor(
                    out=accv[:, :, 3:4], in0=accv[:, :, 3:4],
                    in1=bgf[:].rearrange("p (j o) -> p j o", o=1), op=op.add)

                # foreground mask output
                fgt = outp.tile([128, BLK], f32, tag="fgt")
                nc.vector.tensor_scalar(
                    out=fgt[:].rearrange("p (j o) -> p j o", o=1), in0=ip0,
                    scalar1=0, scalar2=None, op0=op.is_ge)
                nc.sync.dma_start(fg_d.ap()[b], fgt[:])
                nc.sync.dma_start(img_d.ap()[b], acc[:])

            pending = None
            for b in range(NBATCH):
                state = emit_prep_gather(b)
                if pending is not None:
                    emit_select(b - 1, *pending)
                pending = state
            emit_select(NBATCH - 1, *pending)

    nc.compile()
    return nc


def _get_nc():
    global _NC_CACHE
    if _NC_CACHE is None:
        _NC_CACHE = _build_nc()
    return _NC_CACHE


def _prep_core_inputs(pix_idxs_n, alphas_n, tab):
    """Host-side layout (pure permutation / dtype casts) for one image."""
    Pk = pix_idxs_n.reshape(K, HW).astype(np.int32)
    A = alphas_n.reshape(K, HW).astype(np.float32)
    # wrapped: idxw[b, 16g+s, c] = Pk[g, b*NB + c*16 + s]
    idxw = (
        Pk.reshape(K, NBATCH, NB16, 16)
        .transpose(1, 0, 3, 2)
        .reshape(NBATCH, 128, NB16)
    )
    # pixel-major: idxp[b, p, j*8+k] = Pk[k, b*NB + j*128 + p]
    idxp = (
        Pk.reshape(K, NBATCH, BLK, 128)
        .transpose(1, 3, 2, 0)
        .reshape(NBATCH, 128, BLK * 8)
    )
    alp = (
        A.reshape(K, NBATCH, BLK, 128)
        .transpose(1, 3, 2, 0)
        .reshape(NBATCH, 128, BLK * 8)
    )
    return {
        "tab": tab,
        "idxw": np.ascontiguousarray(idxw),
        "idxp": np.ascontiguousarray(idxp),
        "alp": np.ascontiguousarray(alp),
    }


def kernel(pix_idxs, alphas, ptclds):
    from concourse import bass_utils

    nc = _get_nc()
    T = np.asarray(ptclds, dtype=np.float32)
    # tab[16g + 4q + ch, e] = T[ch, q*CHUNK + e], identical per group g
    base = T.reshape(C, 4, CHUNK).transpose(1, 0, 2).reshape(16, CHUNK)
    tab = np.ascontiguousarray(np.tile(base, (8, 1)))

    pix_idxs = np.asarray(pix_idxs)
    alphas = np.asarray(alphas)
    in_maps = [
        _prep_core_inputs(pix_idxs[n], alphas[n], tab) for n in range(N)
    ]
    res = bass_utils.run_bass_kernel_spmd(nc, in_maps, core_ids=list(range(8)))

    images = np.empty((N, C, H, W), np.float32)
    masks = np.empty((N, H, W), bool)
    for n in range(N):
        img = res.results[n]["img"]  # [NBATCH, 128, BLK*4]
        fg = res.results[n]["fg"]    # [NBATCH, 128, BLK]
        img_full = (
            img.reshape(NBATCH, 128, BLK, C)
            .transpose(3, 0, 2, 1)
            .reshape(C, H, W)
        )
        images[n] = img_full
        masks[n] = (
            fg.reshape(NBATCH, 128, BLK)
            .transpose(0, 2, 1)
            .reshape(H, W)
        ) > 0.5
    return images, masks
